# revision 6
# baseline (speedup 1.0000x reference)
"""Trainium2 Bass kernel for nn_CrossAttention (conv-cat cross attention).

Sharding: pure data-parallel over batch B=8 across the 8 NeuronCores
(one batch element per core, no collectives). Each core runs the full
conv + qkv + attention + proj + residual pipeline for its batch element
in bf16 (fp32 residual added exactly at the end).

Self-contained: hardcodes shapes B=8, N=1024, H=W=32, C=1024, nh=8, hd=128.
"""

import sys

if "/opt/trn_rl_repo" not in sys.path:
    sys.path.insert(0, "/opt/trn_rl_repo")

import numpy as np
from ml_dtypes import bfloat16

import concourse.bass as bass
import concourse.mybir as mybir
import concourse.tile as tile
from concourse.bass import ds
from concourse.bass_utils import run_bass_kernel_spmd
from concourse.vector_clock import ScopedClock

P = 128
N = 1024          # channels of conv output == attention sequence length
C = 1024          # H*W == feature dim
NH = 8
HD = C // NH      # 128 == P
SCALE = float(HD) ** -0.5
BF = mybir.dt.bfloat16
F32 = mybir.dt.float32
Copy = mybir.ActivationFunctionType.Copy
Exp = mybir.ActivationFunctionType.Exp
ADD = mybir.AluOpType.add
MULT = mybir.AluOpType.mult


class PatchedTileContext(tile.TileContext):
    """Walrus in this container rejects >1 sync-wait on the kernel-tail
    Drain; split the global-clock waits into single-wait NOPs instead."""

    def _drain_and_barrier(self, tick_clock, wait_clock):
        probe = self.nc.sync.nop(nofuse=True)
        wait_clock.add_sem_waits(
            probe.ins, ScopedClock({None: tick_clock.global_clock})
        )
        waits = list(probe.ins.sync_info.on_wait or [])
        probe.ins.sync_info = mybir.SyncInfo(on_wait=[], on_update=[])
        for w in waits:
            n = self.nc.sync.nop(nofuse=True)
            n.ins.sync_info = mybir.SyncInfo(on_wait=[w], on_update=[])
        self.nc.sync.drain()
        self.nc.all_engine_barrier()
        popped = self.nc._tile_sem_poison_stack.pop()
        assert popped is self._sem_poison
        self.nc.clear_and_free_semaphores(list(self.sems.allocated().values()))
        self.nc.all_engine_barrier()


def _split_multi_waits(nc: bass.Bass) -> None:
    """Walrus in this container allows at most ONE sync-wait per
    instruction. Hoist extra waits into single-wait NOPs emitted just
    before the instruction on the same engine."""
    import copy as _copy

    probe = nc.sync.nop(nofuse=True)
    tmpl = probe.ins
    # Remove the probe from whichever block it was appended to.
    for fn in nc.m.functions:
        for bb in fn.blocks:
            il = list(bb.instructions)
            if il and il[-1].name == tmpl.name:
                bb.instructions = il[:-1]

    counter = 0
    for fn in nc.m.functions:
        for bb in fn.blocks:
            il = list(bb.instructions)
            if not any(
                i.sync_info and len(i.sync_info.on_wait or []) > 1 for i in il
            ):
                continue
            new = []
            for inst in il:
                si = inst.sync_info
                if si and si.on_wait and len(si.on_wait) > 1:
                    waits = list(si.on_wait)
                    for w in waits[:-1]:
                        nn = _copy.copy(tmpl)
                        nn.name = f"{tmpl.name}-sw{counter}"
                        counter += 1
                        nn.engine = inst.engine
                        nn.sync_info = mybir.SyncInfo(on_wait=[w], on_update=[])
                        new.append(nn)
                    inst.sync_info = mybir.SyncInfo(
                        on_wait=[waits[-1]], on_update=list(si.on_update or [])
                    )
                new.append(inst)
            bb.instructions = new


def build_program(debug: bool = False) -> bass.Bass:
    nc = bass.Bass()

    x1b = nc.declare_dram_parameter("x1b", [N, C], BF, isOutput=False)
    x2b = nc.declare_dram_parameter("x2b", [N, C], BF, isOutput=False)
    x1t = nc.declare_dram_parameter("x1t", [C, N], BF, isOutput=False)
    x1f = nc.declare_dram_parameter("x1f", [N, C], F32, isOutput=False)
    wc = nc.declare_dram_parameter("wc", [9, 2 * N, N], BF, isOutput=False)
    wqd = nc.declare_dram_parameter("wq", [C, C], BF, isOutput=False)
    wkd = nc.declare_dram_parameter("wk", [C, C], BF, isOutput=False)
    wvd = nc.declare_dram_parameter("wv", [C, C], BF, isOutput=False)
    wpd = nc.declare_dram_parameter("wp", [C, C], BF, isOutput=False)
    cbd = nc.declare_dram_parameter("cb", [N], F32, isOutput=False)
    pbd = nc.declare_dram_parameter("pb", [C], F32, isOutput=False)
    outd = nc.declare_dram_parameter("out", [N, C], F32, isOutput=True)

    dbg = {}
    if debug:
        for name in ("xcatT", "qT", "kT", "v", "xattnT"):
            dbg[name] = nc.declare_dram_parameter(
                "dbg_" + name, [P, 8, 1024], F32, isOutput=True
            )

    with PatchedTileContext(nc) as tc:
        with (
            tc.tile_pool(name="persist", bufs=1) as persist,
            tc.tile_pool(name="wmov", bufs=6) as pw_mov,
            tc.tile_pool(name="wstat", bufs=2) as pw_stat,
            tc.tile_pool(name="ps", bufs=8, space="PSUM") as pps,
        ):
            # ---- persistent SBUF tensors ----
            xcatT = persist.tile([P, 8, 1024], BF)   # [c_spat, n_chan] transposed conv out
            qT = persist.tile([P, 8, 1024], BF)      # [c_feat, n]
            kT = persist.tile([P, 8, 1024], BF)      # [c_feat, m]
            vN = persist.tile([P, 8, 1024], BF)      # [m, c_feat] natural
            xattnT = persist.tile([P, 8, 1024], BF)  # [c_feat, n]
            cb_s = persist.tile([P, 1024], F32)      # conv_b bcast over partitions
            pb_s = persist.tile([P, 1024], F32)      # proj_b bcast over partitions
            ones_bf = persist.tile([P, 1], BF)
            ones_f1 = persist.tile([1, P], F32)

            nc.vector.memset(ones_bf[:], 1.0)
            nc.vector.memset(ones_f1[:], 1.0)
            cb_ap = cbd[:]
            pb_ap = pbd[:]
            nc.sync.dma_start(
                cb_s[:],
                bass.AP(tensor=cb_ap.tensor, offset=cb_ap.offset,
                        ap=[[0, P], [1, N]]),
            )
            nc.sync.dma_start(
                pb_s[:],
                bass.AP(tensor=pb_ap.tensor, offset=pb_ap.offset,
                        ap=[[0, P], [1, C]]),
            )

            with tc.tile_pool(name="early", bufs=1) as early:
                # padded conv input: 16 ci-tiles of [34, 34] (x1: 0-7, x2: 8-15)
                xpad = early.tile([P, 16, 34, 34], BF)
                x1t_s = early.tile([P, 8, 1024], BF)

                nc.vector.memset(xpad[:], 0.0)
                x1b_r = x1b.rearrange("(a p) (h w) -> p a h w", p=P, h=32)
                x2b_r = x2b.rearrange("(a p) (h w) -> p a h w", p=P, h=32)
                for t in range(8):
                    nc.sync.dma_start(
                        xpad[:, t, ds(1, 32), ds(1, 32)], x1b_r[:, t]
                    )
                    nc.sync.dma_start(
                        xpad[:, 8 + t, ds(1, 32), ds(1, 32)], x2b_r[:, t]
                    )
                x1t_r = x1t.rearrange("(a p) n -> p a n", p=P)
                for t in range(8):
                    nc.sync.dma_start(x1t_s[:, t, :], x1t_r[:, t])

                # ---- phase Q: qT[co, n] = wqT.T @ x1T, scaled ----
                wq_r = wqd.rearrange("(a p) n -> p a n", p=P)
                for cot in range(8):
                    wq_t = pw_stat.tile([P, 8, P], BF, tag="wstat")
                    nc.sync.dma_start(wq_t[:], wq_r[:, :, ds(cot * P, P)])
                    for nh_ in range(2):
                        q_ps = pps.tile([P, 512], F32, tag="ps")
                        for cit in range(8):
                            nc.tensor.matmul(
                                q_ps[:],
                                wq_t[:, cit, :],
                                x1t_s[:, cit, ds(nh_ * 512, 512)],
                                start=(cit == 0),
                                stop=(cit == 7),
                            )
                        nc.scalar.activation(
                            qT[:, cot, ds(nh_ * 512, 512)], q_ps[:], Copy,
                            scale=SCALE,
                        )

                # ---- phase CONV: xcatT[hw, co] += xshift.T @ wc ----
                for coh in range(2):
                    for mg in range(2):
                        cps = [pps.tile([P, 512], F32, tag="ps", name=f"cps{coh}_{mg}_{i}") for i in range(4)]
                        ki = 0
                        for off in range(9):
                            dy, dx = off // 3, off % 3
                            for cit in range(16):
                                wblk = pw_mov.tile([P, 512], BF, tag="wmov")
                                nc.sync.dma_start(
                                    wblk[:],
                                    wc[off, ds(cit * P, P), ds(coh * 512, 512)],
                                )
                                for mi in range(4):
                                    m = mg * 4 + mi
                                    for r in range(4):
                                        nc.tensor.matmul(
                                            cps[mi][ds(32 * r, 32), :],
                                            xpad[:, cit, 4 * m + dy + r, ds(dx, 32)],
                                            wblk[:],
                                            start=(ki == 0),
                                            stop=(ki == 143),
                                            tile_position=(0, 32 * r),
                                        )
                                ki += 1
                        for mi in range(4):
                            m = mg * 4 + mi
                            nc.vector.tensor_tensor(
                                xcatT[:, m, ds(coh * 512, 512)],
                                cps[mi][:],
                                cb_s[:, ds(coh * 512, 512)],
                                ADD,
                            )

            # ---- phase K: kT[co, m] = wkT.T @ xcatT ----
            wk_r = wkd.rearrange("(a p) n -> p a n", p=P)
            for cot in range(8):
                wk_t = pw_stat.tile([P, 8, P], BF, tag="wstat")
                nc.sync.dma_start(wk_t[:], wk_r[:, :, ds(cot * P, P)])
                for nh_ in range(2):
                    k_ps = pps.tile([P, 512], F32, tag="ps")
                    for cit in range(8):
                        nc.tensor.matmul(
                            k_ps[:],
                            wk_t[:, cit, :],
                            xcatT[:, cit, ds(nh_ * 512, 512)],
                            start=(cit == 0),
                            stop=(cit == 7),
                        )
                    nc.scalar.activation(
                        kT[:, cot, ds(nh_ * 512, 512)], k_ps[:], Copy
                    )

            # ---- phase V: v[m, co] = xcatT.T @ wvT  (natural layout) ----
            for coh in range(2):
                for mg in range(2):
                    vps = [pps.tile([P, 512], F32, tag="ps", name=f"vps{coh}_{mg}_{i}") for i in range(4)]
                    for cit in range(8):
                        wvblk = pw_mov.tile([P, 512], BF, tag="wmov")
                        nc.sync.dma_start(
                            wvblk[:], wvd[ds(cit * P, P), ds(coh * 512, 512)]
                        )
                        for mi in range(4):
                            mt = mg * 4 + mi
                            nc.tensor.matmul(
                                vps[mi][:],
                                xcatT[:, cit, ds(mt * P, P)],
                                wvblk[:],
                                start=(cit == 0),
                                stop=(cit == 7),
                            )
                    for mi in range(4):
                        mt = mg * 4 + mi
                        nc.scalar.activation(
                            vN[:, mt, ds(coh * 512, 512)], vps[mi][:], Copy
                        )

            # ---- phase ATTN (per head) ----
            with (
                tc.tile_pool(name="et", bufs=2) as p_et,
                tc.tile_pool(name="rsp", bufs=2) as p_rs,
                tc.tile_pool(name="rbp", bufs=2) as p_rb,
            ):
                for h in range(8):
                    e = p_et.tile([P, 8, 1024], BF, tag="eT")
                    # scoresT[m, n] = kT_h.T @ qT_h ; exp
                    for mt in range(8):
                        for nh_ in range(2):
                            sps = pps.tile([P, 512], F32, tag="ps")
                            nc.tensor.matmul(
                                sps[:],
                                kT[:, h, ds(mt * P, P)],
                                qT[:, h, ds(nh_ * 512, 512)],
                                start=True,
                                stop=True,
                            )
                            nc.scalar.activation(
                                e[:, mt, ds(nh_ * 512, 512)], sps[:], Exp
                            )
                    # denominator S[n] = sum_m e[m, n]; rs = 1/S
                    rs = p_rs.tile([1, 1024], F32, tag="rs")
                    for nh_ in range(2):
                        ssum = pps.tile([1, 512], F32, tag="ps")
                        for mt in range(8):
                            nc.tensor.matmul(
                                ssum[:],
                                ones_bf[:],
                                e[:, mt, ds(nh_ * 512, 512)],
                                start=(mt == 0),
                                stop=(mt == 7),
                            )
                        nc.vector.reciprocal(rs[:, ds(nh_ * 512, 512)], ssum[:])
                    # broadcast rs across partitions via K=1 outer product
                    rb = p_rb.tile([P, 1024], F32, tag="rb")
                    for nh_ in range(2):
                        rbps = pps.tile([P, 512], F32, tag="ps")
                        nc.tensor.matmul(
                            rbps[:],
                            ones_f1[:],
                            rs[:, ds(nh_ * 512, 512)],
                            start=True,
                            stop=True,
                        )
                        nc.scalar.activation(
                            rb[:, ds(nh_ * 512, 512)], rbps[:], Copy
                        )
                    # PV: xattnT_h[d, n] = sum_m v_h[m, d].T e[m, n], normalized
                    for nh_ in range(2):
                        pvps = pps.tile([P, 512], F32, tag="ps")
                        for mt in range(8):
                            nc.tensor.matmul(
                                pvps[:],
                                vN[:, mt, ds(h * P, P)],
                                e[:, mt, ds(nh_ * 512, 512)],
                                start=(mt == 0),
                                stop=(mt == 7),
                            )
                        nc.vector.tensor_tensor(
                            xattnT[:, h, ds(nh_ * 512, 512)],
                            pvps[:],
                            rb[:, ds(nh_ * 512, 512)],
                            MULT,
                        )

            # ---- phase PROJ + residual ----
            x1f_r = x1f.rearrange("(a p) n -> p a n", p=P)
            with tc.tile_pool(name="x1p", bufs=4) as p_x1:
                for ng in range(2):
                    xbs = []
                    for ni in range(4):
                        nt = ng * 4 + ni
                        xb = p_x1.tile([P, 1024], F32, tag="xb")
                        nc.sync.dma_start(xb[:], x1f_r[:, nt])
                        nc.vector.tensor_tensor(xb[:], xb[:], pb_s[:], ADD)
                        xbs.append(xb)
                    for coh in range(2):
                        pp4 = [pps.tile([P, 512], F32, tag="ps", name=f"pp{ng}_{coh}_{i}") for i in range(4)]
                        for cit in range(8):
                            wpblk = pw_mov.tile([P, 512], BF, tag="wmov")
                            nc.sync.dma_start(
                                wpblk[:], wpd[ds(cit * P, P), ds(coh * 512, 512)]
                            )
                            for ni in range(4):
                                nt = ng * 4 + ni
                                nc.tensor.matmul(
                                    pp4[ni][:],
                                    xattnT[:, cit, ds(nt * P, P)],
                                    wpblk[:],
                                    start=(cit == 0),
                                    stop=(cit == 7),
                                )
                        for ni in range(4):
                            nc.vector.tensor_tensor(
                                xbs[ni][:, ds(coh * 512, 512)],
                                pp4[ni][:],
                                xbs[ni][:, ds(coh * 512, 512)],
                                ADD,
                            )
                    out_r = outd.rearrange("(a p) n -> p a n", p=P)
                    for ni in range(4):
                        nt = ng * 4 + ni
                        nc.sync.dma_start(out_r[:, nt], xbs[ni][:])

            if debug:
                with tc.tile_pool(name="dbgp", bufs=2) as p_dbg:
                    for name, sb in (
                        ("xcatT", xcatT), ("qT", qT), ("kT", kT),
                        ("v", vN), ("xattnT", xattnT),
                    ):
                        for t in range(8):
                            db = p_dbg.tile([P, 1024], F32, tag="db")
                            nc.vector.tensor_copy(db[:], sb[:, t, :])
                            nc.sync.dma_start(dbg[name][:, t, :], db[:])

    _split_multi_waits(nc)
    return nc


_PROGRAM_CACHE = {}


def _get_program(debug: bool = False) -> bass.Bass:
    if debug not in _PROGRAM_CACHE:
        _PROGRAM_CACHE[debug] = build_program(debug)
    return _PROGRAM_CACHE[debug]


def _prep_in_maps(x1, x2, conv_w, conv_b, wq, wk, wv, proj_w, proj_b):
    B, Nc, H, W = x1.shape
    Cd = H * W
    x1m = np.asarray(x1, dtype=np.float32).reshape(B, Nc, Cd)
    x2m = np.asarray(x2, dtype=np.float32).reshape(B, Nc, Cd)
    wc = np.ascontiguousarray(
        np.asarray(conv_w, dtype=np.float32).transpose(2, 3, 1, 0)
    ).reshape(9, 2 * Nc, Nc).astype(bfloat16)
    wqt = np.ascontiguousarray(np.asarray(wq, dtype=np.float32).T).astype(bfloat16)
    wkt = np.ascontiguousarray(np.asarray(wk, dtype=np.float32).T).astype(bfloat16)
    wvt = np.ascontiguousarray(np.asarray(wv, dtype=np.float32).T).astype(bfloat16)
    wpt = np.ascontiguousarray(np.asarray(proj_w, dtype=np.float32).T).astype(bfloat16)
    cb = np.asarray(conv_b, dtype=np.float32)
    pb = np.asarray(proj_b, dtype=np.float32)
    in_maps = []
    for b in range(B):
        in_maps.append({
            "x1b": x1m[b].astype(bfloat16),
            "x2b": x2m[b].astype(bfloat16),
            "x1t": np.ascontiguousarray(x1m[b].T).astype(bfloat16),
            "x1f": np.ascontiguousarray(x1m[b]),
            "wc": wc,
            "wq": wqt,
            "wk": wkt,
            "wv": wvt,
            "wp": wpt,
            "cb": cb,
            "pb": pb,
        })
    return in_maps


def kernel_run(inputs: dict, trace: bool = False, debug: bool = False):
    """Run the kernel; returns (output [8,1024,32,32] f32, BassKernelResults)."""
    in_maps = _prep_in_maps(**inputs)
    nc = _get_program(debug)
    if trace:
        _install_trace_hook()
    res = run_bass_kernel_spmd(nc, in_maps, list(range(8)), trace=trace)
    B = len(in_maps)
    out = np.stack([r["out"] for r in res.results]).reshape(B, N, 32, 32)
    return np.ascontiguousarray(out, dtype=np.float32), res


def kernel(**inputs) -> np.ndarray:
    out, _ = kernel_run(inputs, trace=False)
    return out


def _install_trace_hook():
    """Synthesize the missing antenv.axon_hooks so trace=True works."""
    import types

    if "antenv.axon_hooks" in sys.modules:
        return
    import antenv
    from trn_agent_boot.trn_boot import _ntff_profile_via_ctypes

    mod = types.ModuleType("antenv.axon_hooks")
    hook = _ntff_profile_via_ctypes("/opt/axon/libaxon_pjrt.so")
    mod.get_axon_ntff_profile_hook = lambda: hook
    mod.set_axon_ntff_profile_hook = lambda h: None
    sys.modules["antenv.axon_hooks"] = mod
    antenv.axon_hooks = mod


# revision 8
# speedup vs baseline: 1.0720x; 1.0720x over previous
"""Trainium2 Bass kernel for nn_CrossAttention (conv-cat cross attention).

Sharding: pure data-parallel over batch B=8 across the 8 NeuronCores
(one batch element per core, no collectives). Each core runs the full
conv + qkv + attention + proj + residual pipeline for its batch element
in bf16 (fp32 residual added exactly at the end).

Self-contained: hardcodes shapes B=8, N=1024, H=W=32, C=1024, nh=8, hd=128.
"""

import sys

if "/opt/trn_rl_repo" not in sys.path:
    sys.path.insert(0, "/opt/trn_rl_repo")

import numpy as np
from ml_dtypes import bfloat16

import concourse.bass as bass
import concourse.mybir as mybir
import concourse.tile as tile
from concourse.bass import ds
from concourse.bass_utils import run_bass_kernel_spmd
from concourse.vector_clock import ScopedClock

P = 128
N = 1024          # channels of conv output == attention sequence length
C = 1024          # H*W == feature dim
PADW = 34
PAD2 = PADW * PADW  # 1156
NH = 8
HD = C // NH      # 128 == P
SCALE = float(HD) ** -0.5
BF = mybir.dt.bfloat16
F32 = mybir.dt.float32
Copy = mybir.ActivationFunctionType.Copy
Exp = mybir.ActivationFunctionType.Exp
ADD = mybir.AluOpType.add
MULT = mybir.AluOpType.mult


class PatchedTileContext(tile.TileContext):
    """Walrus in this container rejects >1 sync-wait on the kernel-tail
    Drain; split the global-clock waits into single-wait NOPs instead."""

    def _drain_and_barrier(self, tick_clock, wait_clock):
        probe = self.nc.sync.nop(nofuse=True)
        wait_clock.add_sem_waits(
            probe.ins, ScopedClock({None: tick_clock.global_clock})
        )
        waits = list(probe.ins.sync_info.on_wait or [])
        probe.ins.sync_info = mybir.SyncInfo(on_wait=[], on_update=[])
        for w in waits:
            n = self.nc.sync.nop(nofuse=True)
            n.ins.sync_info = mybir.SyncInfo(on_wait=[w], on_update=[])
        self.nc.sync.drain()
        self.nc.all_engine_barrier()
        popped = self.nc._tile_sem_poison_stack.pop()
        assert popped is self._sem_poison
        self.nc.clear_and_free_semaphores(list(self.sems.allocated().values()))
        self.nc.all_engine_barrier()


def _split_multi_waits(nc: bass.Bass) -> None:
    """Walrus in this container allows at most ONE sync-wait per
    instruction. Hoist extra waits into single-wait NOPs emitted just
    before the instruction on the same engine."""
    import copy as _copy

    probe = nc.sync.nop(nofuse=True)
    tmpl = probe.ins
    for fn in nc.m.functions:
        for bb in fn.blocks:
            il = list(bb.instructions)
            if il and il[-1].name == tmpl.name:
                bb.instructions = il[:-1]

    counter = 0
    for fn in nc.m.functions:
        for bb in fn.blocks:
            il = list(bb.instructions)
            if not any(
                i.sync_info and len(i.sync_info.on_wait or []) > 1 for i in il
            ):
                continue
            new = []
            for inst in il:
                si = inst.sync_info
                if si and si.on_wait and len(si.on_wait) > 1:
                    waits = list(si.on_wait)
                    for w in waits[:-1]:
                        nn = _copy.copy(tmpl)
                        nn.name = f"{tmpl.name}-sw{counter}"
                        counter += 1
                        nn.engine = inst.engine
                        nn.sync_info = mybir.SyncInfo(on_wait=[w], on_update=[])
                        new.append(nn)
                    inst.sync_info = mybir.SyncInfo(
                        on_wait=[waits[-1]], on_update=list(si.on_update or [])
                    )
                new.append(inst)
            bb.instructions = new


def build_program(debug: bool = False) -> bass.Bass:
    nc = bass.Bass()

    x1p = nc.declare_dram_parameter("x1p", [N, PAD2], BF, isOutput=False)
    x2p = nc.declare_dram_parameter("x2p", [N, PAD2], BF, isOutput=False)
    x1t = nc.declare_dram_parameter("x1t", [C, N], BF, isOutput=False)
    x1f = nc.declare_dram_parameter("x1f", [N, C], F32, isOutput=False)
    wc = nc.declare_dram_parameter("wc", [9, 2 * N, N], BF, isOutput=False)
    wqd = nc.declare_dram_parameter("wq", [8, P, 8, P], BF, isOutput=False)
    wkd = nc.declare_dram_parameter("wk", [8, P, 8, P], BF, isOutput=False)
    wvd = nc.declare_dram_parameter("wv", [C, C], BF, isOutput=False)
    wpd = nc.declare_dram_parameter("wp", [C, C], BF, isOutput=False)
    cbd = nc.declare_dram_parameter("cb", [N], F32, isOutput=False)
    pbd = nc.declare_dram_parameter("pb", [C], F32, isOutput=False)
    outd = nc.declare_dram_parameter("out", [N, C], F32, isOutput=True)

    dbg = {}
    if debug:
        for name in ("xcatT", "qT", "kT", "v", "xattnT"):
            dbg[name] = nc.declare_dram_parameter(
                "dbg_" + name, [P, 8, 1024], F32, isOutput=True
            )

    with PatchedTileContext(nc) as tc:
        with (
            tc.tile_pool(name="persist", bufs=1) as persist,
            tc.tile_pool(name="wmov", bufs=6) as pw_mov,
            tc.tile_pool(name="wstat", bufs=2) as pw_stat,
            tc.tile_pool(name="ps", bufs=4, space="PSUM") as pps,
        ):
            # ---- persistent SBUF tensors ----
            xcatT = persist.tile([P, 8, 1024], BF)   # [c_spat, n_chan] conv out, transposed
            qT = persist.tile([P, 8, 1024], BF)      # [c_feat, n]
            kT = persist.tile([P, 8, 1024], BF)      # [c_feat, m]
            vN = persist.tile([P, 8, 1024], BF)      # [m, c_feat] natural
            xattnT = persist.tile([P, 8, 1024], BF)  # [c_feat, n]
            cb_s = persist.tile([P, 1024], F32)      # conv_b bcast over partitions
            pb_s = persist.tile([P, 1024], F32)      # proj_b bcast over partitions
            ones_bf = persist.tile([P, 1], BF)
            ones_f1 = persist.tile([1, P], F32)

            nc.vector.memset(ones_bf[:], 1.0)
            nc.vector.memset(ones_f1[:], 1.0)
            cb_ap = cbd[:]
            pb_ap = pbd[:]
            nc.sync.dma_start(
                cb_s[:],
                bass.AP(tensor=cb_ap.tensor, offset=cb_ap.offset,
                        ap=[[0, P], [1, N]]),
            )
            nc.sync.dma_start(
                pb_s[:],
                bass.AP(tensor=pb_ap.tensor, offset=pb_ap.offset,
                        ap=[[0, P], [1, C]]),
            )

            with tc.tile_pool(name="early", bufs=1) as early:
                # padded conv input: 16 ci-tiles of 34x34 (x1: 0-7, x2: 8-15)
                xpad = early.tile([P, 16, PAD2], BF)
                x1t_s = early.tile([P, 8, 1024], BF)

                x1p_r = x1p.rearrange("(a p) s -> p a s", p=P)
                x2p_r = x2p.rearrange("(a p) s -> p a s", p=P)
                for t in range(8):
                    nc.sync.dma_start(xpad[:, t, :], x1p_r[:, t])
                    nc.sync.dma_start(xpad[:, 8 + t, :], x2p_r[:, t])
                x1t_r = x1t.rearrange("(a p) n -> p a n", p=P)
                for t in range(8):
                    nc.sync.dma_start(x1t_s[:, t, :], x1t_r[:, t])

                # ---- phase Q: qT[co, n] = wqT.T @ x1T, scaled ----
                for cot in range(8):
                    wq_t = pw_stat.tile([P, 8, P], BF, tag="wstat")
                    nc.sync.dma_start(wq_t[:], wqd[cot])
                    q_ps = pps.tile([P, 1024], F32, tag="ps")
                    for nh_ in range(2):
                        for cit in range(8):
                            nc.tensor.matmul(
                                q_ps[:, ds(nh_ * 512, 512)],
                                wq_t[:, cit, :],
                                x1t_s[:, cit, ds(nh_ * 512, 512)],
                                start=(cit == 0),
                                stop=(cit == 7),
                            )
                    nc.scalar.activation(qT[:, cot, :], q_ps[:], Copy,
                                         scale=SCALE)

                # ---- phase CONV: xcatT[hw, co] += xshift.T @ wc ----
                # 2 passes over hw-tile groups of 4; each pass streams wc once
                # as [128, 1024] blocks; psum [128,1024] (2 banks) per hw-tile.
                for mg in range(2):
                    cps = [
                        pps.tile([P, 1024], F32, tag="ps", name=f"cps{mg}_{i}")
                        for i in range(4)
                    ]
                    ki = 0
                    for off in range(9):
                        dy, dx = off // 3, off % 3
                        for cit in range(16):
                            wblk = pw_mov.tile([P, 1024], BF, tag="wmov")
                            nc.sync.dma_start(wblk[:], wc[off, ds(cit * P, P), :])
                            for mi in range(4):
                                m = mg * 4 + mi
                                for r in range(4):
                                    row = 4 * m + dy + r
                                    lhs = xpad[:, cit, ds(row * PADW + dx, 32)]
                                    for coh in range(2):
                                        nc.tensor.matmul(
                                            cps[mi][ds(32 * r, 32),
                                                    ds(coh * 512, 512)],
                                            lhs,
                                            wblk[:, ds(coh * 512, 512)],
                                            start=(ki == 0),
                                            stop=(ki == 143),
                                            tile_position=(0, 32 * r),
                                        )
                            ki += 1
                    for mi in range(4):
                        m = mg * 4 + mi
                        nc.vector.tensor_tensor(
                            xcatT[:, m, :], cps[mi][:], cb_s[:], ADD
                        )

            # ---- phase K: kT[co, m] = wkT.T @ xcatT ----
            for cot in range(8):
                wk_t = pw_stat.tile([P, 8, P], BF, tag="wstat")
                nc.sync.dma_start(wk_t[:], wkd[cot])
                k_ps = pps.tile([P, 1024], F32, tag="ps")
                for nh_ in range(2):
                    for cit in range(8):
                        nc.tensor.matmul(
                            k_ps[:, ds(nh_ * 512, 512)],
                            wk_t[:, cit, :],
                            xcatT[:, cit, ds(nh_ * 512, 512)],
                            start=(cit == 0),
                            stop=(cit == 7),
                        )
                nc.scalar.activation(kT[:, cot, :], k_ps[:], Copy)

            # ---- phase V: v[m, co] = xcatT.T @ wvT  (natural layout) ----
            for mg in range(2):
                vps = [
                    pps.tile([P, 1024], F32, tag="ps", name=f"vps{mg}_{i}")
                    for i in range(4)
                ]
                for cit in range(8):
                    wvblk = pw_mov.tile([P, 1024], BF, tag="wmov")
                    nc.sync.dma_start(wvblk[:], wvd[ds(cit * P, P), :])
                    for mi in range(4):
                        mt = mg * 4 + mi
                        for coh in range(2):
                            nc.tensor.matmul(
                                vps[mi][:, ds(coh * 512, 512)],
                                xcatT[:, cit, ds(mt * P, P)],
                                wvblk[:, ds(coh * 512, 512)],
                                start=(cit == 0),
                                stop=(cit == 7),
                            )
                for mi in range(4):
                    mt = mg * 4 + mi
                    nc.scalar.activation(vN[:, mt, :], vps[mi][:], Copy)

            # ---- phase ATTN (per head) ----
            with (
                tc.tile_pool(name="et", bufs=2) as p_et,
                tc.tile_pool(name="rsp", bufs=2) as p_rs,
                tc.tile_pool(name="rbp", bufs=2) as p_rb,
            ):
                for h in range(8):
                    e = p_et.tile([P, 8, 1024], BF, tag="eT")
                    # scoresT[m, n] = kT_h.T @ qT_h ; exp
                    for mt in range(8):
                        sps = pps.tile([P, 1024], F32, tag="ps")
                        for nh_ in range(2):
                            nc.tensor.matmul(
                                sps[:, ds(nh_ * 512, 512)],
                                kT[:, h, ds(mt * P, P)],
                                qT[:, h, ds(nh_ * 512, 512)],
                                start=True,
                                stop=True,
                            )
                        nc.scalar.activation(e[:, mt, :], sps[:], Exp)
                    # denominator S[n] = sum_m e[m, n]; rs = 1/S
                    rs = p_rs.tile([1, 1024], F32, tag="rs")
                    ssum = pps.tile([1, 1024], F32, tag="ps")
                    for nh_ in range(2):
                        for mt in range(8):
                            nc.tensor.matmul(
                                ssum[:, ds(nh_ * 512, 512)],
                                ones_bf[:],
                                e[:, mt, ds(nh_ * 512, 512)],
                                start=(mt == 0),
                                stop=(mt == 7),
                            )
                    nc.vector.reciprocal(rs[:], ssum[:])
                    # broadcast rs across partitions via K=1 outer product
                    rb = p_rb.tile([P, 1024], F32, tag="rb")
                    rbps = pps.tile([P, 1024], F32, tag="ps")
                    for nh_ in range(2):
                        nc.tensor.matmul(
                            rbps[:, ds(nh_ * 512, 512)],
                            ones_f1[:],
                            rs[:, ds(nh_ * 512, 512)],
                            start=True,
                            stop=True,
                        )
                    nc.scalar.activation(rb[:], rbps[:], Copy)
                    # PV: xattnT_h[d, n] = sum_m v_h[m, d].T e[m, n], normalized
                    pvps = pps.tile([P, 1024], F32, tag="ps")
                    for nh_ in range(2):
                        for mt in range(8):
                            nc.tensor.matmul(
                                pvps[:, ds(nh_ * 512, 512)],
                                vN[:, mt, ds(h * P, P)],
                                e[:, mt, ds(nh_ * 512, 512)],
                                start=(mt == 0),
                                stop=(mt == 7),
                            )
                    nc.vector.tensor_tensor(
                        xattnT[:, h, :], pvps[:], rb[:], MULT
                    )

            # ---- phase PROJ + residual ----
            x1f_r = x1f.rearrange("(a p) n -> p a n", p=P)
            out_r = outd.rearrange("(a p) n -> p a n", p=P)
            with tc.tile_pool(name="x1pool", bufs=4) as p_x1:
                for ng in range(2):
                    xbs = []
                    for ni in range(4):
                        nt = ng * 4 + ni
                        xb = p_x1.tile([P, 1024], F32, tag="xb")
                        nc.sync.dma_start(xb[:], x1f_r[:, nt])
                        nc.vector.tensor_tensor(xb[:], xb[:], pb_s[:], ADD)
                        xbs.append(xb)
                    pp4 = [
                        pps.tile([P, 1024], F32, tag="ps", name=f"pp{ng}_{i}")
                        for i in range(4)
                    ]
                    for cit in range(8):
                        wpblk = pw_mov.tile([P, 1024], BF, tag="wmov")
                        nc.sync.dma_start(wpblk[:], wpd[ds(cit * P, P), :])
                        for ni in range(4):
                            nt = ng * 4 + ni
                            for coh in range(2):
                                nc.tensor.matmul(
                                    pp4[ni][:, ds(coh * 512, 512)],
                                    xattnT[:, cit, ds(nt * P, P)],
                                    wpblk[:, ds(coh * 512, 512)],
                                    start=(cit == 0),
                                    stop=(cit == 7),
                                )
                    for ni in range(4):
                        nc.vector.tensor_tensor(
                            xbs[ni][:], pp4[ni][:], xbs[ni][:], ADD
                        )
                    for ni in range(4):
                        nt = ng * 4 + ni
                        nc.sync.dma_start(out_r[:, nt], xbs[ni][:])

            if debug:
                with tc.tile_pool(name="dbgp", bufs=2) as p_dbg:
                    for name, sb in (
                        ("xcatT", xcatT), ("qT", qT), ("kT", kT),
                        ("v", vN), ("xattnT", xattnT),
                    ):
                        for t in range(8):
                            db = p_dbg.tile([P, 1024], F32, tag="db")
                            nc.vector.tensor_copy(db[:], sb[:, t, :])
                            nc.sync.dma_start(dbg[name][:, t, :], db[:])

    _split_multi_waits(nc)
    return nc


_PROGRAM_CACHE = {}


def _get_program(debug: bool = False) -> bass.Bass:
    if debug not in _PROGRAM_CACHE:
        _PROGRAM_CACHE[debug] = build_program(debug)
    return _PROGRAM_CACHE[debug]


def _prep_in_maps(x1, x2, conv_w, conv_b, wq, wk, wv, proj_w, proj_b):
    B, Nc, H, W = x1.shape
    Cd = H * W
    x1m = np.asarray(x1, dtype=np.float32).reshape(B, Nc, Cd)
    x2m = np.asarray(x2, dtype=np.float32).reshape(B, Nc, Cd)

    def pad_img(xm):
        # [Nc, H*W] f32 -> [Nc, 34*34] bf16 zero-padded
        out = np.zeros((Nc, PADW, PADW), dtype=bfloat16)
        out[:, 1:33, 1:33] = xm.reshape(Nc, H, W).astype(bfloat16)
        return out.reshape(Nc, PAD2)

    def tile_stat(w):
        # W [co, ci] -> W.T tiled [cot, p, a, c] with
        # wh[cot, p, a, c] = W.T[a*128+p, cot*128+c]
        wt = np.asarray(w, dtype=np.float32).T
        return np.ascontiguousarray(
            wt.reshape(8, P, 8, P).transpose(2, 1, 0, 3)
        ).astype(bfloat16)

    wc = np.ascontiguousarray(
        np.asarray(conv_w, dtype=np.float32).transpose(2, 3, 1, 0)
    ).reshape(9, 2 * Nc, Nc).astype(bfloat16)
    wqh = tile_stat(wq)
    wkh = tile_stat(wk)
    wvt = np.ascontiguousarray(np.asarray(wv, dtype=np.float32).T).astype(bfloat16)
    wpt = np.ascontiguousarray(np.asarray(proj_w, dtype=np.float32).T).astype(bfloat16)
    cb = np.asarray(conv_b, dtype=np.float32)
    pb = np.asarray(proj_b, dtype=np.float32)
    in_maps = []
    for b in range(B):
        in_maps.append({
            "x1p": pad_img(x1m[b]),
            "x2p": pad_img(x2m[b]),
            "x1t": np.ascontiguousarray(x1m[b].T).astype(bfloat16),
            "x1f": np.ascontiguousarray(x1m[b]),
            "wc": wc,
            "wq": wqh,
            "wk": wkh,
            "wv": wvt,
            "wp": wpt,
            "cb": cb,
            "pb": pb,
        })
    return in_maps


def kernel_run(inputs: dict, trace: bool = False, debug: bool = False):
    """Run the kernel; returns (output [8,1024,32,32] f32, BassKernelResults)."""
    in_maps = _prep_in_maps(**inputs)
    nc = _get_program(debug)
    if trace:
        _install_trace_hook()
    res = run_bass_kernel_spmd(nc, in_maps, list(range(8)), trace=trace)
    B = len(in_maps)
    out = np.stack([r["out"] for r in res.results]).reshape(B, N, 32, 32)
    return np.ascontiguousarray(out, dtype=np.float32), res


def kernel(**inputs) -> np.ndarray:
    out, _ = kernel_run(inputs, trace=False)
    return out


def _install_trace_hook():
    """Synthesize the missing antenv.axon_hooks so trace=True works."""
    import types

    if "antenv.axon_hooks" in sys.modules:
        return
    import antenv
    from trn_agent_boot.trn_boot import _ntff_profile_via_ctypes

    mod = types.ModuleType("antenv.axon_hooks")
    hook = _ntff_profile_via_ctypes("/opt/axon/libaxon_pjrt.so")
    mod.get_axon_ntff_profile_hook = lambda: hook
    mod.set_axon_ntff_profile_hook = lambda h: None
    sys.modules["antenv.axon_hooks"] = mod
    antenv.axon_hooks = mod
